# revision 52
# baseline (speedup 1.0000x reference)
"""BiasedMHA Trainium2 kernel (v2).

Problem: B=4, N=1024, FEAT=512, H=8 multihead attention with additive bias and
boolean mask, softmax over the key dim, output projection.

Sharding (8 cores): core c handles batch b = c//2 and head-half hh = c%2
(4 heads, full 1024 query rows). Each core computes K/Q/V projections for its
4 heads, biased attention, and its heads' partial output projection (two
head-pair partials). Host sums the 4 partials per batch and adds bo + bv@Wo.T
(exact: softmax rows sum to 1).

Key structure vs v1:
  - exp(bias) with mask folded (masked -> 0) is computed on HOST; the device
    computes et = exp(scores) * eb so the ACT engine reads scores straight
    from PSUM and the DVE does a cheap all-f16 SBUF multiply (2x_1p mode)
    instead of a slow f32 PSUM add.
  - score matmuls contract over HD=64: the two heads of a pair sit in
    partition halves [0:64)/[64:128) of shared KT/QT tiles, so their matmuls
    run concurrently in separate PE row-groups (tile_position auto-derived).
  - denominator via a ones-column in V' (row 64 of the PV output); recip on
    DVE, broadcast across partitions on GpSimd, normalize-multiply on DVE.
  - output projection contracts head pairs stacked to 128 rows; the two
    head-pair partials go back to HBM in f16 and are summed on host.
"""

import numpy as np

import concourse.bass as bass
import concourse.mybir as mybir
import concourse.tile as tile
from concourse import bacc
from concourse.bass_utils import run_bass_kernel_spmd

# All activation funcs used here (Exp, Copy) live in the
# natural_log_exp_and_others table set; restrict the insertion pass to it so
# the kernel does exactly one ACT_TABLE_LOAD.
_orig_get_tables = bacc.get_activation_tables


def _one_table(arch):
    t = _orig_get_tables(arch)
    return {k: (v if k == "natural_log_exp_and_others" else set())
            for k, v in t.items()}


bacc.get_activation_tables = _one_table


B, N, FEAT, H = 4, 1024, 512, 8
HD = FEAT // H  # 64
SCALE = HD ** -0.5
N_CORES = 8
NH = 4    # heads per core
NHP = 2   # head pairs per core
NFC = 4   # 128-row feature chunks (projection contract dim)
NJT = 8   # 128-row key chunks
VW = 66   # V' per-head stride: 64 dims + ones col + 1 pad

F32 = mybir.dt.float32
F16 = mybir.dt.float16
I16 = mybir.dt.int16
# f16 Schraudolph exp: exp(x) ~ bitcast_f16(int16(EXPA*x + EXPB))
EXPA = 1024.0 / float(np.log(2.0))
EXPB = 15360.0 - 59.3
AF = mybir.ActivationFunctionType
ALU = mybir.AluOpType

_CACHE = {}


def _build():
    nc = bacc.Bacc("TRN2", target_bir_lowering=False, debug=False)

    ndT = nc.dram_tensor("ndT", [FEAT, N], F16, kind="ExternalInput").ap()
    wk4 = nc.dram_tensor("wk4", [128, NFC * 256], F16, kind="ExternalInput").ap()
    wq4 = nc.dram_tensor("wq4", [128, NFC * 256], F16, kind="ExternalInput").ap()
    wv4 = nc.dram_tensor("wv4", [128, NFC * 256], F16, kind="ExternalInput").ap()
    wo4 = nc.dram_tensor("wo4", [128, NHP * FEAT], F16, kind="ExternalInput").ap()
    bkq = nc.dram_tensor("bkq", [128, 4], F32, kind="ExternalInput").ap()
    # eb[h][p, jt*N + i] = exp(bias + mask) for local head h, key j=jt*128+p
    ebd = nc.dram_tensor("ebd", [NH, 128, NJT * N], F16, kind="ExternalInput").ap()
    outs = [
        nc.dram_tensor(f"out{hp}", [N, FEAT], F16, kind="ExternalOutput").ap()
        for hp in range(NHP)
    ]

    with tile.TileContext(nc) as tc:
        with (
            tc.tile_pool(name="persist", bufs=1) as persist,
            tc.tile_pool(name="work", bufs=2) as work,
            tc.tile_pool(name="ps_st", bufs=4, space="PSUM") as ps_st,
            tc.tile_pool(name="ps_out", bufs=2, space="PSUM") as ps_out,
        ):
            # ---- persistent SBUF tiles
            ndc = [[persist.tile([128, 512], F16, tag=f"nd{fc}{jh}",
                                 name=f"nd{fc}{jh}")
                    for jh in range(2)] for fc in range(NFC)]
            wk_a = persist.tile([128, 256], F16, tag="wk_a")
            wk_b = persist.tile([128, 3 * 256], F16, tag="wk_b")
            wq_a = persist.tile([128, 256], F16, tag="wq_a")
            wq_b = persist.tile([128, 3 * 256], F16, tag="wq_b")
            wv = persist.tile([128, NFC * 256], F16, tag="wv")
            wo = persist.tile([128, NHP * FEAT], F16, tag="wo")
            bkq_sb = persist.tile([128, 4], F32, tag="bkq")
            # KT/QT split per j/i-half so the first scores start after the
            # first-half projections only (deps are tile-granular)
            KT = [[persist.tile([128, 512], F16, tag=f"kt{t}{jh}",
                                name=f"kt{t}{jh}") for jh in range(2)]
                  for t in range(NHP)]
            QT = [[persist.tile([128, 512], F16, tag=f"qt{t}{jh}",
                                name=f"qt{t}{jh}") for jh in range(2)]
                  for t in range(NHP)]
            # V' in four tiles (2 jt each) so each PV chunk waits only for
            # its own slice of the V projection
            V4 = [persist.tile([128, 2 * NH * VW], F16, tag=f"v4{q}",
                               name=f"v4{q}") for q in range(4)]
            v4r = [v.rearrange("p (j c) -> p j c", c=NH * VW) for v in V4]
            eb = [[persist.tile([128, 4 * N], F16, tag=f"eb{h}{hf}",
                                name=f"eb{h}{hf}")
                   for hf in range(2)] for h in range(NH)]
            OTn = [persist.tile([128, N], F16, tag=f"otn{hp}", name=f"otn{hp}")
                   for hp in range(NHP)]
            ones16 = persist.tile([128, 64], F16, tag="ones16")

            def wslice(a, b_, fc, t):
                # stationary [128, 128] slice for K/Q proj tile t, chunk fc
                w = a if fc == 0 else b_
                off = 0 if fc == 0 else (fc - 1) * 256
                return w[:, off + t * 128: off + (t + 1) * 128]

            # ---- input DMAs: first-half projection inputs first (they gate
            # the first scores), then second half, then eb per head in
            # consumption order
            nc.sync.dma_start(out=bkq_sb, in_=bkq)
            nc.scalar.dma_start(out=wk_a, in_=wk4[:, 0:256])
            nc.sync.dma_start(out=ndc[0][0], in_=ndT[0:128, 0:512])
            nc.scalar.dma_start(out=wq_a, in_=wq4[:, 0:256])
            nc.sync.dma_start(out=ndc[1][0], in_=ndT[128:256, 0:512])
            nc.scalar.dma_start(out=wk_b, in_=wk4[:, 256:1024])
            nc.sync.dma_start(out=ndc[2][0], in_=ndT[256:384, 0:512])
            nc.scalar.dma_start(out=wq_b, in_=wq4[:, 256:1024])
            nc.sync.dma_start(out=ndc[3][0], in_=ndT[384:512, 0:512])
            nc.scalar.dma_start(out=wv, in_=wv4)
            nc.scalar.dma_start(out=ndc[0][1], in_=ndT[0:128, 512:1024])
            nc.sync.dma_start(out=ndc[1][1], in_=ndT[128:256, 512:1024])
            nc.scalar.dma_start(out=ndc[2][1], in_=ndT[256:384, 512:1024])
            nc.sync.dma_start(out=ndc[3][1], in_=ndT[384:512, 512:1024])
            nc.scalar.dma_start(out=eb[0][0], in_=ebd[0][:, 0:4096])
            nc.sync.dma_start(out=eb[0][1], in_=ebd[0][:, 4096:8192])
            nc.scalar.dma_start(out=eb[1][0], in_=ebd[1][:, 0:4096])
            nc.sync.dma_start(out=eb[1][1], in_=ebd[1][:, 4096:8192])
            nc.sync.dma_start(out=wo, in_=wo4)
            nc.sync.dma_start(out=eb[2][0], in_=ebd[2][:, 0:4096])
            nc.sync.dma_start(out=eb[2][1], in_=ebd[2][:, 4096:8192])
            nc.sync.dma_start(out=eb[3][0], in_=ebd[3][:, 0:4096])
            nc.sync.dma_start(out=eb[3][1], in_=ebd[3][:, 4096:8192])

            nc.gpsimd.memset(ones16, 1.0)
            # ones columns of V' (col 64 of each head block)
            for q in range(4):
                for h in range(NH):
                    nc.gpsimd.memset(
                        v4r[q][:, :, h * VW + 64:h * VW + 65], 1.0)

            # ---- projections (first-half for both pairs, then second half)
            def emit_kqproj(t, jh):
                kq = ps_st.tile([128, 512], F32, tag="st1", name=f"kp{t}{jh}")
                qq = ps_st.tile([128, 512], F32, tag="st1", name=f"qp{t}{jh}")
                for fc in range(NFC):
                    nc.tensor.matmul(
                        kq, wslice(wk_a, wk_b, fc, t), ndc[fc][jh],
                        start=(fc == 0), stop=(fc == NFC - 1),
                    )
                nc.vector.tensor_scalar_add(KT[t][jh], kq, bkq_sb[:, t:t + 1])
                for fc in range(NFC):
                    nc.tensor.matmul(
                        qq, wslice(wq_a, wq_b, fc, t), ndc[fc][jh],
                        start=(fc == 0), stop=(fc == NFC - 1),
                    )
                nc.vector.tensor_scalar(
                    QT[t][jh], qq, bkq_sb[:, 2 + t:3 + t], SCALE,
                    op0=ALU.add, op1=ALU.mult,
                )

            def emit_vproj_pair(jp):
                ps = ps_st.tile([128, 512], F32, tag="st1", name=f"vp{jp}")
                for k in range(2):
                    jt = 2 * jp + k
                    sub = ps[:, k * 256:(k + 1) * 256]
                    for fc in range(NFC):
                        nc.tensor.matmul(
                            sub,
                            ndc[fc][jt // 4][:, (jt % 4) * 128:
                                             (jt % 4 + 1) * 128],
                            wv.rearrange("p (c d) -> p c d", d=256)[:, fc, :],
                            start=(fc == 0), stop=(fc == NFC - 1),
                        )
                    nc.vector.tensor_copy(
                        v4r[jp].rearrange(
                            "p j (h c) -> p j h c", c=VW)[:, k, :, 0:64],
                        sub.rearrange("p (h c) -> p h c", c=64),
                    )

            # ---- output projection chunk for one head pair (2 row chunks)
            def emit_oproj_chunk(hp, icp, tail=False):
                fps = [ps_st.tile([128, 512], F32, tag="st1",
                                  name=f"fp{hp}_{icp}_{k}") for k in range(2)]
                for k in range(2):
                    ic = icp * 2 + k
                    nc.tensor.matmul(
                        fps[k],
                        OTn[hp][:, ic * 128:(ic + 1) * 128],
                        wo[:, hp * FEAT:(hp + 1) * FEAT],
                        start=True, stop=True,
                    )
                for k in range(2):
                    ic = icp * 2 + k
                    osb = work.tile([128, 512], F16, tag="osb", bufs=4,
                                    name=f"osb{hp}_{ic}")
                    if tail and k == 0:
                        nc.scalar.activation(osb, fps[k], AF.Copy)
                    else:
                        nc.vector.tensor_copy(osb, fps[k])
                    nc.sync.dma_start(
                        out=outs[hp][ic * 128:(ic + 1) * 128, :], in_=osb
                    )

            # ---- attention pair, software-pipelined in 2-jt windows so the
            # in-order PE queue always alternates scores (feeding ACT) with
            # PV work, and `filler` work slots into the PE slack
            def emit_pair(hp, filler, prelude=None, inline_norm=False):
                hA, hB = 2 * hp, 2 * hp + 1
                es = [work.tile([128, NJT * N], F16, tag="es", bufs=2,
                                name=f"es{h}") for h in (hA, hB)]
                et = [work.tile([128, NJT * N], F16, tag="et", bufs=2,
                                name=f"et{h}") for h in (hA, hB)]
                etr = [e.rearrange("p (j i) -> p j i", i=N) for e in et]
                outps = [ps_out.tile([128, 1024], F32, tag="outp",
                                     name=f"outp{h}") for h in (hA, hB)]

                def emit_scores(jt, ics=(0, 1)):
                    for ic in ics:
                        st = [ps_st.tile([128, 512], F32, tag="st1",
                                         name=f"st{hp}_{jt}_{ic}_{i}")
                              for i in range(2)]
                        for idx in range(2):
                            po = idx * 64
                            nc.tensor.matmul(
                                st[idx],
                                KT[hp][jt // 4][po:po + 64,
                                                (jt % 4) * 128:
                                                (jt % 4 + 1) * 128],
                                QT[hp][ic][po:po + 64, :],
                                start=True, stop=True,
                            )
                        for idx in range(2):
                            nc.scalar.activation(
                                es[idx][:, jt * N + ic * 512:
                                        jt * N + ic * 512 + 512],
                                st[idx], AF.Exp
                            )

                def emit_mult(jt):
                    for idx, h in enumerate((hA, hB)):
                        ebs = eb[h][jt // 4][:, (jt % 4) * N:
                                             (jt % 4) * N + N]
                        ess = es[idx][:, jt * N:(jt + 1) * N]
                        ets = et[idx][:, jt * N:(jt + 1) * N]
                        nc.vector.tensor_mul(ets, ess, ebs)

                def emit_pv_jt(jt, idx):
                    h = (hA, hB)[idx]
                    for ic in range(2):
                        nc.tensor.matmul(
                            outps[idx][0:65, ic * 512:(ic + 1) * 512],
                            v4r[jt // 2][:, jt % 2, h * VW:h * VW + 65],
                            etr[idx][:, jt, ic * 512:(ic + 1) * 512],
                            start=(jt == 0),
                            stop=(jt == NJT - 1),
                            skip_group_check=True,
                        )

                # normalization, deferred + split: pre = PSUM eviction and
                # the Ln/Exp reciprocal (ACT/DVE); post = broadcast matmuls
                # and the normalize multiply (PE/DVE). The caller slots these
                # into the next pair's windows so neither the ACT nor the PE
                # queue head-blocks at pair boundaries.
                state = {}

                def norm_pre(idx, h):
                    outp = outps[idx]
                    otd = work.tile([64, N], F32, tag="otd", name=f"otd{h}")
                    nc.vector.tensor_copy(otd, outp[0:64, :])
                    lnv = work.tile([65, N], F32, tag="lnv", name=f"lnv{h}")
                    nc.scalar.activation(lnv[64:65, :], outp[64:65, :], AF.Ln)
                    rec16 = work.tile([65, N], F16, tag="rec16",
                                      name=f"rec16{h}")
                    nc.scalar.activation(rec16[64:65, :], lnv[64:65, :],
                                         AF.Exp, scale=-1.0)
                    state[idx] = (otd, rec16)

                def norm_post(idx, h):
                    otd, rec16 = state[idx]
                    rbcs = [ps_st.tile([128, 512], F32, tag="st1",
                                       name=f"rbc{h}_{ic}") for ic in range(2)]
                    for ic in range(2):
                        nc.tensor.matmul(
                            rbcs[ic][0:64, :],
                            ones16[64:65, :],
                            rec16[64:65, ic * 512:(ic + 1) * 512],
                            start=True, stop=True,
                        )
                        nc.vector.tensor_mul(
                            OTn[hp][idx * 64:idx * 64 + 64,
                                    ic * 512:(ic + 1) * 512],
                            otd[:, ic * 512:(ic + 1) * 512],
                            rbcs[ic][0:64, :],
                        )

                if prelude is not None:
                    # ic0 scores of the first two key chunks depend only on
                    # the first projection tiles -> the exp stream starts
                    # while the second projection (prelude) still runs
                    emit_scores(0, ics=(0,))
                    emit_scores(1, ics=(0,))
                    prelude()
                    emit_scores(0, ics=(1,))
                    emit_scores(1, ics=(1,))
                for w in range(4):
                    for k in range(2):
                        if not (prelude is not None and w == 0):
                            emit_scores(2 * w + k)
                    if filler:
                        f = filler.pop(0)
                        if f is not None:
                            f()
                    if w >= 1:
                        for jt in (2 * (w - 1), 2 * (w - 1) + 1):
                            for idx in range(2):
                                emit_pv_jt(jt, idx)
                    emit_mult(2 * w)
                    emit_mult(2 * w + 1)
                for jt in (6, 7):
                    for idx in range(2):
                        emit_pv_jt(jt, idx)
                        if inline_norm and jt == 7:
                            norm_pre(idx, (hA, hB)[idx])
                if inline_norm:
                    for idx in range(2):
                        norm_post(idx, (hA, hB)[idx])


                if inline_norm:
                    return []
                return [lambda: (norm_pre(0, hA), norm_pre(1, hB)),
                        lambda: (norm_post(0, hA), norm_post(1, hB))]

            # ---- schedule: minimal pre-pair projections; V'/remaining
            # projections/previous norms/oproj ride the pair windows
            emit_kqproj(0, 0)

            norm0 = emit_pair(0, [
                lambda: emit_vproj_pair(0),
                lambda: emit_vproj_pair(1),
                lambda: emit_vproj_pair(2),
                lambda: (emit_vproj_pair(3), emit_kqproj(1, 0),
                         emit_kqproj(1, 1)),
            ], prelude=lambda: emit_kqproj(0, 1))
            emit_pair(1, [
                None,
                norm0[0],
                lambda: (norm0[1](), emit_oproj_chunk(0, 0)),
                lambda: (emit_oproj_chunk(0, 1), emit_oproj_chunk(0, 2)),
            ], inline_norm=True)
            emit_oproj_chunk(0, 3)
            for icp in range(4):
                emit_oproj_chunk(1, icp, tail=True)

    nc.compile()
    return nc


def _prep_inputs(ndata, attn_bias, attn_mask, Wq, bq, Wk, bk, Wv, bv, Wo, bo):
    ndata = np.asarray(ndata, dtype=np.float32)
    attn_bias = np.asarray(attn_bias, dtype=np.float32)
    attn_mask = np.asarray(attn_mask)
    Wq, Wk, Wv, Wo = (np.asarray(w, dtype=np.float32) for w in (Wq, Wk, Wv, Wo))
    bq, bk, bv, bo = (np.asarray(v, dtype=np.float32) for v in (bq, bk, bv, bo))

    ebf = np.where(attn_mask, np.float32(0.0), np.exp(attn_bias))  # [B,N,N,H]

    wqT = Wq.T.astype(np.float16)  # [in_f, out_f]
    wkT = Wk.T.astype(np.float16)
    wvT = Wv.T.astype(np.float16)
    woT = Wo.T.astype(np.float16)

    in_maps = []
    for core in range(N_CORES):
        b, hh = core // 2, core % 2
        cs = hh * 256  # feature-col offset of this core's 4 heads
        wk4 = np.ascontiguousarray(
            wkT[:, cs:cs + 256].reshape(4, 128, 256).transpose(1, 0, 2)
            .reshape(128, 1024))
        wq4 = np.ascontiguousarray(
            wqT[:, cs:cs + 256].reshape(4, 128, 256).transpose(1, 0, 2)
            .reshape(128, 1024))
        wv4 = np.ascontiguousarray(
            wvT[:, cs:cs + 256].reshape(4, 128, 256).transpose(1, 0, 2)
            .reshape(128, 1024))
        wo4 = np.ascontiguousarray(
            woT[cs:cs + 256, :].reshape(2, 128, 512).transpose(1, 0, 2)
            .reshape(128, 1024))
        bkq = np.ascontiguousarray(np.concatenate(
            [bk[cs:cs + 256].reshape(2, 128).T,
             bq[cs:cs + 256].reshape(2, 128).T], axis=1)).astype(np.float32)
        arr = ebf[b][:, :, hh * 4:hh * 4 + 4]        # [i, j, hl]
        arr = arr.transpose(2, 1, 0)                 # [hl, j, i]
        arr = arr.reshape(4, 8, 128, N).transpose(0, 2, 1, 3)  # [hl,p,jt,i]
        ebc = np.ascontiguousarray(arr.reshape(4, 128, 8192).astype(np.float16))
        in_maps.append({
            "ndT": np.ascontiguousarray(ndata[b].T.astype(np.float16)),
            "wk4": wk4, "wq4": wq4, "wv4": wv4, "wo4": wo4,
            "bkq": bkq, "ebd": ebc,
        })
    return in_maps


def kernel(ndata, attn_bias, attn_mask, Wq, bq, Wk, bk, Wv, bv, Wo, bo,
           _trace=False):
    if "nc" not in _CACHE:
        _CACHE["nc"] = _build()
    nc = _CACHE["nc"]
    in_maps = _prep_inputs(ndata, attn_bias, attn_mask, Wq, bq, Wk, bk, Wv, bv,
                           Wo, bo)
    res = run_bass_kernel_spmd(nc, in_maps, list(range(N_CORES)), trace=_trace)
    _CACHE["last_res"] = res
    const = (np.asarray(bo, dtype=np.float32)
             + np.asarray(bv, dtype=np.float32)
             @ np.asarray(Wo, dtype=np.float32).T)
    full = np.empty((B, N, FEAT), dtype=np.float32)
    for b in range(B):
        acc = np.zeros((N, FEAT), dtype=np.float32)
        for hh in range(2):
            r = res.results[2 * b + hh]
            acc += r["out0"].astype(np.float32)
            acc += r["out1"].astype(np.float32)
        full[b] = acc + const[None, :]
    return full


# revision 53
# speedup vs baseline: 1.1147x; 1.1147x over previous
"""BiasedMHA Trainium2 kernel (v2).

Problem: B=4, N=1024, FEAT=512, H=8 multihead attention with additive bias and
boolean mask, softmax over the key dim, output projection.

Sharding (8 cores): core c handles batch b = c//2 and head-half hh = c%2
(4 heads, full 1024 query rows). Each core computes K/Q/V projections for its
4 heads, biased attention, and its heads' partial output projection (two
head-pair partials). Host sums the 4 partials per batch and adds bo + bv@Wo.T
(exact: softmax rows sum to 1).

Key structure vs v1:
  - exp(bias) with mask folded (masked -> 0) is computed on HOST; the device
    computes et = exp(scores) * eb so the ACT engine reads scores straight
    from PSUM and the DVE does a cheap all-f16 SBUF multiply (2x_1p mode)
    instead of a slow f32 PSUM add.
  - score matmuls contract over HD=64: the two heads of a pair sit in
    partition halves [0:64)/[64:128) of shared KT/QT tiles, so their matmuls
    run concurrently in separate PE row-groups (tile_position auto-derived).
  - denominator via a ones-column in V' (row 64 of the PV output); recip on
    DVE, broadcast across partitions on GpSimd, normalize-multiply on DVE.
  - output projection contracts head pairs stacked to 128 rows; the two
    head-pair partials go back to HBM in f16 and are summed on host.
"""

import numpy as np

import concourse.bass as bass
import concourse.mybir as mybir
import concourse.tile as tile
from concourse import bacc
from concourse.bass_utils import run_bass_kernel_spmd

# All activation funcs used here (Exp, Copy) live in the
# natural_log_exp_and_others table set; restrict the insertion pass to it so
# the kernel does exactly one ACT_TABLE_LOAD.
_orig_get_tables = bacc.get_activation_tables


def _one_table(arch):
    t = _orig_get_tables(arch)
    return {k: (v if k == "natural_log_exp_and_others" else set())
            for k, v in t.items()}


bacc.get_activation_tables = _one_table


B, N, FEAT, H = 4, 1024, 512, 8
HD = FEAT // H  # 64
SCALE = HD ** -0.5
N_CORES = 8
NH = 4    # heads per core
NHP = 2   # head pairs per core
NFC = 4   # 128-row feature chunks (projection contract dim)
NJT = 8   # 128-row key chunks
VW = 66   # V' per-head stride: 64 dims + ones col + 1 pad

F32 = mybir.dt.float32
F16 = mybir.dt.float16
I16 = mybir.dt.int16
# f16 Schraudolph exp: exp(x) ~ bitcast_f16(int16(EXPA*x + EXPB))
EXPA = 1024.0 / float(np.log(2.0))
EXPB = 15360.0 - 59.3
AF = mybir.ActivationFunctionType
ALU = mybir.AluOpType

_CACHE = {}


def _build():
    nc = bacc.Bacc("TRN2", target_bir_lowering=False, debug=False)

    ndT = nc.dram_tensor("ndT", [FEAT, N], F16, kind="ExternalInput").ap()
    wk4 = nc.dram_tensor("wk4", [128, NFC * 256], F16, kind="ExternalInput").ap()
    wq4 = nc.dram_tensor("wq4", [128, NFC * 256], F16, kind="ExternalInput").ap()
    wv4 = nc.dram_tensor("wv4", [128, NFC * 256], F16, kind="ExternalInput").ap()
    wo4 = nc.dram_tensor("wo4", [128, NHP * FEAT], F16, kind="ExternalInput").ap()
    bkq = nc.dram_tensor("bkq", [128, 4], F32, kind="ExternalInput").ap()
    # eb[h][p, jt*N + i] = exp(bias + mask) for local head h, key j=jt*128+p
    ebd = nc.dram_tensor("ebd", [NH, 128, NJT * N], F16, kind="ExternalInput").ap()
    outs = [
        nc.dram_tensor(f"out{hp}", [N, FEAT], F16, kind="ExternalOutput").ap()
        for hp in range(NHP)
    ]

    with tile.TileContext(nc) as tc:
        with (
            tc.tile_pool(name="persist", bufs=1) as persist,
            tc.tile_pool(name="work", bufs=2) as work,
            tc.tile_pool(name="ps_st", bufs=4, space="PSUM") as ps_st,
            tc.tile_pool(name="ps_out", bufs=2, space="PSUM") as ps_out,
        ):
            # ---- persistent SBUF tiles
            ndc = [[persist.tile([128, 512], F16, tag=f"nd{fc}{jh}",
                                 name=f"nd{fc}{jh}")
                    for jh in range(2)] for fc in range(NFC)]
            wk_a = persist.tile([128, 256], F16, tag="wk_a")
            wk_b = persist.tile([128, 3 * 256], F16, tag="wk_b")
            wq_a = persist.tile([128, 256], F16, tag="wq_a")
            wq_b = persist.tile([128, 3 * 256], F16, tag="wq_b")
            wv = persist.tile([128, NFC * 256], F16, tag="wv")
            wo = persist.tile([128, NHP * FEAT], F16, tag="wo")
            bkq_sb = persist.tile([128, 4], F32, tag="bkq")
            # KT/QT split per j/i-half so the first scores start after the
            # first-half projections only (deps are tile-granular)
            KT = [[persist.tile([128, 512], F16, tag=f"kt{t}{jh}",
                                name=f"kt{t}{jh}") for jh in range(2)]
                  for t in range(NHP)]
            QT = [[persist.tile([128, 512], F16, tag=f"qt{t}{jh}",
                                name=f"qt{t}{jh}") for jh in range(2)]
                  for t in range(NHP)]
            # V' in four tiles (2 jt each) so each PV chunk waits only for
            # its own slice of the V projection
            V4 = [persist.tile([128, 2 * NH * VW], F16, tag=f"v4{q}",
                               name=f"v4{q}") for q in range(4)]
            v4r = [v.rearrange("p (j c) -> p j c", c=NH * VW) for v in V4]
            eb = [[persist.tile([128, 4 * N], F16, tag=f"eb{h}{hf}",
                                name=f"eb{h}{hf}")
                   for hf in range(2)] for h in range(NH)]
            OTn = [persist.tile([128, N], F16, tag=f"otn{hp}", name=f"otn{hp}")
                   for hp in range(NHP)]
            ones16 = persist.tile([128, 64], F16, tag="ones16")

            def wslice(a, b_, fc, t):
                # stationary [128, 128] slice for K/Q proj tile t, chunk fc
                w = a if fc == 0 else b_
                off = 0 if fc == 0 else (fc - 1) * 256
                return w[:, off + t * 128: off + (t + 1) * 128]

            # ---- input DMAs: first-half projection inputs first (they gate
            # the first scores), then second half, then eb per head in
            # consumption order
            nc.sync.dma_start(out=bkq_sb, in_=bkq)
            nc.scalar.dma_start(out=wk_a, in_=wk4[:, 0:256])
            nc.sync.dma_start(out=ndc[0][0], in_=ndT[0:128, 0:512])
            nc.scalar.dma_start(out=wq_a, in_=wq4[:, 0:256])
            nc.sync.dma_start(out=ndc[1][0], in_=ndT[128:256, 0:512])
            nc.scalar.dma_start(out=wk_b, in_=wk4[:, 256:1024])
            nc.sync.dma_start(out=ndc[2][0], in_=ndT[256:384, 0:512])
            nc.scalar.dma_start(out=wq_b, in_=wq4[:, 256:1024])
            nc.sync.dma_start(out=ndc[3][0], in_=ndT[384:512, 0:512])
            nc.scalar.dma_start(out=wv, in_=wv4)
            nc.scalar.dma_start(out=ndc[0][1], in_=ndT[0:128, 512:1024])
            nc.sync.dma_start(out=ndc[1][1], in_=ndT[128:256, 512:1024])
            nc.scalar.dma_start(out=ndc[2][1], in_=ndT[256:384, 512:1024])
            nc.sync.dma_start(out=ndc[3][1], in_=ndT[384:512, 512:1024])
            nc.scalar.dma_start(out=eb[0][0], in_=ebd[0][:, 0:4096])
            nc.sync.dma_start(out=eb[0][1], in_=ebd[0][:, 4096:8192])
            nc.scalar.dma_start(out=eb[1][0], in_=ebd[1][:, 0:4096])
            nc.sync.dma_start(out=eb[1][1], in_=ebd[1][:, 4096:8192])
            nc.sync.dma_start(out=wo, in_=wo4)
            nc.sync.dma_start(out=eb[2][0], in_=ebd[2][:, 0:4096])
            nc.sync.dma_start(out=eb[2][1], in_=ebd[2][:, 4096:8192])
            nc.sync.dma_start(out=eb[3][0], in_=ebd[3][:, 0:4096])
            nc.sync.dma_start(out=eb[3][1], in_=ebd[3][:, 4096:8192])

            nc.gpsimd.memset(ones16, 1.0)
            # ones columns of V' (col 64 of each head block)
            for q in range(4):
                for h in range(NH):
                    nc.gpsimd.memset(
                        v4r[q][:, :, h * VW + 64:h * VW + 65], 1.0)

            # ---- projections (first-half for both pairs, then second half)
            def emit_kqproj(t, jh):
                kq = ps_st.tile([128, 512], F32, tag="st1", name=f"kp{t}{jh}")
                qq = ps_st.tile([128, 512], F32, tag="st1", name=f"qp{t}{jh}")
                for fc in range(NFC):
                    nc.tensor.matmul(
                        kq, wslice(wk_a, wk_b, fc, t), ndc[fc][jh],
                        start=(fc == 0), stop=(fc == NFC - 1),
                    )
                nc.vector.tensor_scalar_add(KT[t][jh], kq, bkq_sb[:, t:t + 1])
                for fc in range(NFC):
                    nc.tensor.matmul(
                        qq, wslice(wq_a, wq_b, fc, t), ndc[fc][jh],
                        start=(fc == 0), stop=(fc == NFC - 1),
                    )
                nc.vector.tensor_scalar(
                    QT[t][jh], qq, bkq_sb[:, 2 + t:3 + t], SCALE,
                    op0=ALU.add, op1=ALU.mult,
                )

            def emit_vproj_pair(jp):
                ps = ps_st.tile([128, 512], F32, tag="st1", name=f"vp{jp}")
                for k in range(2):
                    jt = 2 * jp + k
                    sub = ps[:, k * 256:(k + 1) * 256]
                    for fc in range(NFC):
                        nc.tensor.matmul(
                            sub,
                            ndc[fc][jt // 4][:, (jt % 4) * 128:
                                             (jt % 4 + 1) * 128],
                            wv.rearrange("p (c d) -> p c d", d=256)[:, fc, :],
                            start=(fc == 0), stop=(fc == NFC - 1),
                        )
                    nc.vector.tensor_copy(
                        v4r[jp].rearrange(
                            "p j (h c) -> p j h c", c=VW)[:, k, :, 0:64],
                        sub.rearrange("p (h c) -> p h c", c=64),
                    )

            # ---- output projection chunk for one head pair (2 row chunks)
            def emit_oproj_chunk(hp, icp, tail=False):
                fps = [ps_st.tile([128, 512], F32, tag="st1",
                                  name=f"fp{hp}_{icp}_{k}") for k in range(2)]
                for k in range(2):
                    ic = icp * 2 + k
                    nc.tensor.matmul(
                        fps[k],
                        OTn[hp][:, ic * 128:(ic + 1) * 128],
                        wo[:, hp * FEAT:(hp + 1) * FEAT],
                        start=True, stop=True,
                    )
                for k in range(2):
                    ic = icp * 2 + k
                    osb = work.tile([128, 512], F16, tag="osb", bufs=4,
                                    name=f"osb{hp}_{ic}")
                    if tail and k == 0:
                        nc.scalar.activation(osb, fps[k], AF.Copy)
                    else:
                        nc.vector.tensor_copy(osb, fps[k])
                    nc.sync.dma_start(
                        out=outs[hp][ic * 128:(ic + 1) * 128, :], in_=osb
                    )

            # ---- attention pair, software-pipelined in 2-jt windows so the
            # in-order PE queue always alternates scores (feeding ACT) with
            # PV work, and `filler` work slots into the PE slack
            def emit_pair(hp, filler, prelude=None):
                hA, hB = 2 * hp, 2 * hp + 1
                es = [work.tile([128, NJT * N], F16, tag="es", bufs=2,
                                name=f"es{h}") for h in (hA, hB)]
                et = [work.tile([128, NJT * N], F16, tag="et", bufs=2,
                                name=f"et{h}") for h in (hA, hB)]
                etr = [e.rearrange("p (j i) -> p j i", i=N) for e in et]
                outps = [ps_out.tile([128, 1024], F32, tag="outp",
                                     name=f"outp{h}") for h in (hA, hB)]

                def emit_scores(jt, ics=(0, 1)):
                    for ic in ics:
                        st = [ps_st.tile([128, 512], F32, tag="st1",
                                         name=f"st{hp}_{jt}_{ic}_{i}")
                              for i in range(2)]
                        for idx in range(2):
                            po = idx * 64
                            nc.tensor.matmul(
                                st[idx],
                                KT[hp][jt // 4][po:po + 64,
                                                (jt % 4) * 128:
                                                (jt % 4 + 1) * 128],
                                QT[hp][ic][po:po + 64, :],
                                start=True, stop=True,
                            )
                        for idx in range(2):
                            nc.scalar.activation(
                                es[idx][:, jt * N + ic * 512:
                                        jt * N + ic * 512 + 512],
                                st[idx], AF.Exp
                            )

                def emit_mult(jp):
                    for idx, h in enumerate((hA, hB)):
                        ebs = eb[h][jp // 2][:, (jp % 2) * 2048:
                                             (jp % 2) * 2048 + 2048]
                        ess = es[idx][:, jp * 2 * N:(jp * 2 + 2) * N]
                        ets = et[idx][:, jp * 2 * N:(jp * 2 + 2) * N]
                        nc.vector.tensor_mul(ets, ess, ebs)

                def emit_pv(jp):
                    order = ((1, hB), (0, hA)) if jp == 0 else \
                        ((0, hA), (1, hB))
                    for idx, h in order:
                        for ic in range(2):
                            for jt in (2 * jp, 2 * jp + 1):
                                nc.tensor.matmul(
                                    outps[idx][0:65, ic * 512:(ic + 1) * 512],
                                    v4r[jp][:, jt % 2,
                                            h * VW:h * VW + 65],
                                    etr[idx][:, jt, ic * 512:(ic + 1) * 512],
                                    start=(jp == 0 and jt == 0),
                                    stop=(jp == 3 and jt == NJT - 1),
                                    skip_group_check=True,
                                )

                if prelude is not None:
                    # ic0 scores of the first two key chunks depend only on
                    # the first projection tiles -> the exp stream starts
                    # while the second projection (prelude) still runs
                    emit_scores(0, ics=(0,))
                    emit_scores(1, ics=(0,))
                    prelude()
                    emit_scores(0, ics=(1,))
                    emit_scores(1, ics=(1,))
                for w in range(4):
                    for k in range(2):
                        if not (prelude is not None and w == 0):
                            emit_scores(2 * w + k)
                    if filler:
                        f = filler.pop(0)
                        if f is not None:
                            f()
                    if w >= 1:
                        emit_pv(w - 1)
                    emit_mult(w)
                emit_pv(3)

                # normalization, deferred + split: pre = PSUM eviction and
                # the Ln/Exp reciprocal (ACT/DVE); post = broadcast matmuls
                # and the normalize multiply (PE/DVE). The caller slots these
                # into the next pair's windows so neither the ACT nor the PE
                # queue head-blocks at pair boundaries.
                state = {}

                def norm_pre(idx, h):
                    outp = outps[idx]
                    otd = work.tile([64, N], F32, tag="otd", name=f"otd{h}")
                    nc.vector.tensor_copy(otd, outp[0:64, :])
                    lnv = work.tile([65, N], F32, tag="lnv", name=f"lnv{h}")
                    nc.scalar.activation(lnv[64:65, :], outp[64:65, :], AF.Ln)
                    rec16 = work.tile([65, N], F16, tag="rec16",
                                      name=f"rec16{h}")
                    nc.scalar.activation(rec16[64:65, :], lnv[64:65, :],
                                         AF.Exp, scale=-1.0)
                    state[idx] = (otd, rec16)

                def norm_post(idx, h):
                    otd, rec16 = state[idx]
                    rbcs = [ps_st.tile([128, 512], F32, tag="st1",
                                       name=f"rbc{h}_{ic}") for ic in range(2)]
                    for ic in range(2):
                        nc.tensor.matmul(
                            rbcs[ic][0:64, :],
                            ones16[64:65, :],
                            rec16[64:65, ic * 512:(ic + 1) * 512],
                            start=True, stop=True,
                        )
                        nc.vector.tensor_mul(
                            OTn[hp][idx * 64:idx * 64 + 64,
                                    ic * 512:(ic + 1) * 512],
                            otd[:, ic * 512:(ic + 1) * 512],
                            rbcs[ic][0:64, :],
                        )

                return [lambda: (norm_pre(0, hA), norm_pre(1, hB)),
                        lambda: (norm_post(0, hA), norm_post(1, hB))]

            # ---- schedule: minimal pre-pair projections; V'/remaining
            # projections/previous norms/oproj ride the pair windows
            emit_kqproj(0, 0)

            norm0 = emit_pair(0, [
                lambda: emit_vproj_pair(0),
                lambda: emit_vproj_pair(1),
                lambda: emit_vproj_pair(2),
                lambda: (emit_vproj_pair(3), emit_kqproj(1, 0),
                         emit_kqproj(1, 1)),
            ], prelude=lambda: emit_kqproj(0, 1))
            norm1 = emit_pair(1, [
                None,
                norm0[0],
                lambda: (norm0[1](), emit_oproj_chunk(0, 0)),
                lambda: (emit_oproj_chunk(0, 1), emit_oproj_chunk(0, 2)),
            ])
            emit_oproj_chunk(0, 3)
            for nrm in norm1:
                nrm()
            for icp in range(4):
                emit_oproj_chunk(1, icp, tail=True)

    nc.compile()
    return nc


def _prep_inputs(ndata, attn_bias, attn_mask, Wq, bq, Wk, bk, Wv, bv, Wo, bo):
    ndata = np.asarray(ndata, dtype=np.float32)
    attn_bias = np.asarray(attn_bias, dtype=np.float32)
    attn_mask = np.asarray(attn_mask)
    Wq, Wk, Wv, Wo = (np.asarray(w, dtype=np.float32) for w in (Wq, Wk, Wv, Wo))
    bq, bk, bv, bo = (np.asarray(v, dtype=np.float32) for v in (bq, bk, bv, bo))

    ebf = np.where(attn_mask, np.float32(0.0), np.exp(attn_bias))  # [B,N,N,H]

    wqT = Wq.T.astype(np.float16)  # [in_f, out_f]
    wkT = Wk.T.astype(np.float16)
    wvT = Wv.T.astype(np.float16)
    woT = Wo.T.astype(np.float16)

    in_maps = []
    for core in range(N_CORES):
        b, hh = core // 2, core % 2
        cs = hh * 256  # feature-col offset of this core's 4 heads
        wk4 = np.ascontiguousarray(
            wkT[:, cs:cs + 256].reshape(4, 128, 256).transpose(1, 0, 2)
            .reshape(128, 1024))
        wq4 = np.ascontiguousarray(
            wqT[:, cs:cs + 256].reshape(4, 128, 256).transpose(1, 0, 2)
            .reshape(128, 1024))
        wv4 = np.ascontiguousarray(
            wvT[:, cs:cs + 256].reshape(4, 128, 256).transpose(1, 0, 2)
            .reshape(128, 1024))
        wo4 = np.ascontiguousarray(
            woT[cs:cs + 256, :].reshape(2, 128, 512).transpose(1, 0, 2)
            .reshape(128, 1024))
        bkq = np.ascontiguousarray(np.concatenate(
            [bk[cs:cs + 256].reshape(2, 128).T,
             bq[cs:cs + 256].reshape(2, 128).T], axis=1)).astype(np.float32)
        arr = ebf[b][:, :, hh * 4:hh * 4 + 4]        # [i, j, hl]
        arr = arr.transpose(2, 1, 0)                 # [hl, j, i]
        arr = arr.reshape(4, 8, 128, N).transpose(0, 2, 1, 3)  # [hl,p,jt,i]
        ebc = np.ascontiguousarray(arr.reshape(4, 128, 8192).astype(np.float16))
        in_maps.append({
            "ndT": np.ascontiguousarray(ndata[b].T.astype(np.float16)),
            "wk4": wk4, "wq4": wq4, "wv4": wv4, "wo4": wo4,
            "bkq": bkq, "ebd": ebc,
        })
    return in_maps


def kernel(ndata, attn_bias, attn_mask, Wq, bq, Wk, bk, Wv, bv, Wo, bo,
           _trace=False):
    if "nc" not in _CACHE:
        _CACHE["nc"] = _build()
    nc = _CACHE["nc"]
    in_maps = _prep_inputs(ndata, attn_bias, attn_mask, Wq, bq, Wk, bk, Wv, bv,
                           Wo, bo)
    res = run_bass_kernel_spmd(nc, in_maps, list(range(N_CORES)), trace=_trace)
    _CACHE["last_res"] = res
    const = (np.asarray(bo, dtype=np.float32)
             + np.asarray(bv, dtype=np.float32)
             @ np.asarray(Wo, dtype=np.float32).T)
    full = np.empty((B, N, FEAT), dtype=np.float32)
    for b in range(B):
        acc = np.zeros((N, FEAT), dtype=np.float32)
        for hh in range(2):
            r = res.results[2 * b + hh]
            acc += r["out0"].astype(np.float32)
            acc += r["out1"].astype(np.float32)
        full[b] = acc + const[None, :]
    return full


# revision 54
# speedup vs baseline: 1.1218x; 1.0064x over previous
"""BiasedMHA Trainium2 kernel (v2).

Problem: B=4, N=1024, FEAT=512, H=8 multihead attention with additive bias and
boolean mask, softmax over the key dim, output projection.

Sharding (8 cores): core c handles batch b = c//2 and head-half hh = c%2
(4 heads, full 1024 query rows). Each core computes K/Q/V projections for its
4 heads, biased attention, and its heads' partial output projection (two
head-pair partials). Host sums the 4 partials per batch and adds bo + bv@Wo.T
(exact: softmax rows sum to 1).

Key structure vs v1:
  - exp(bias) with mask folded (masked -> 0) is computed on HOST; the device
    computes et = exp(scores) * eb so the ACT engine reads scores straight
    from PSUM and the DVE does a cheap all-f16 SBUF multiply (2x_1p mode)
    instead of a slow f32 PSUM add.
  - score matmuls contract over HD=64: the two heads of a pair sit in
    partition halves [0:64)/[64:128) of shared KT/QT tiles, so their matmuls
    run concurrently in separate PE row-groups (tile_position auto-derived).
  - denominator via a ones-column in V' (row 64 of the PV output); recip on
    DVE, broadcast across partitions on GpSimd, normalize-multiply on DVE.
  - output projection contracts head pairs stacked to 128 rows; the two
    head-pair partials go back to HBM in f16 and are summed on host.
"""

import numpy as np

import concourse.bass as bass
import concourse.mybir as mybir
import concourse.tile as tile
from concourse import bacc
from concourse.bass_utils import run_bass_kernel_spmd

# All activation funcs used here (Exp, Copy) live in the
# natural_log_exp_and_others table set; restrict the insertion pass to it so
# the kernel does exactly one ACT_TABLE_LOAD.
_orig_get_tables = bacc.get_activation_tables


def _one_table(arch):
    t = _orig_get_tables(arch)
    return {k: (v if k == "natural_log_exp_and_others" else set())
            for k, v in t.items()}


bacc.get_activation_tables = _one_table


B, N, FEAT, H = 4, 1024, 512, 8
HD = FEAT // H  # 64
SCALE = HD ** -0.5
N_CORES = 8
NH = 4    # heads per core
NHP = 2   # head pairs per core
NFC = 4   # 128-row feature chunks (projection contract dim)
NJT = 8   # 128-row key chunks
VW = 66   # V' per-head stride: 64 dims + ones col + 1 pad

F32 = mybir.dt.float32
F16 = mybir.dt.float16
I16 = mybir.dt.int16
# f16 Schraudolph exp: exp(x) ~ bitcast_f16(int16(EXPA*x + EXPB))
EXPA = 1024.0 / float(np.log(2.0))
EXPB = 15360.0 - 59.3
AF = mybir.ActivationFunctionType
ALU = mybir.AluOpType

_CACHE = {}


def _build():
    nc = bacc.Bacc("TRN2", target_bir_lowering=False, debug=False)

    ndT = nc.dram_tensor("ndT", [FEAT, N], F16, kind="ExternalInput").ap()
    wk4 = nc.dram_tensor("wk4", [128, NFC * 256], F16, kind="ExternalInput").ap()
    wq4 = nc.dram_tensor("wq4", [128, NFC * 256], F16, kind="ExternalInput").ap()
    wv4 = nc.dram_tensor("wv4", [128, NFC * 256], F16, kind="ExternalInput").ap()
    wo4 = nc.dram_tensor("wo4", [128, NHP * FEAT], F16, kind="ExternalInput").ap()
    bkq = nc.dram_tensor("bkq", [128, 4], F32, kind="ExternalInput").ap()
    # eb[h][p, jt*N + i] = exp(bias + mask) for local head h, key j=jt*128+p
    ebd = nc.dram_tensor("ebd", [NH, 128, NJT * N], F16, kind="ExternalInput").ap()
    outs = [
        nc.dram_tensor(f"out{hp}", [N, FEAT], F16, kind="ExternalOutput").ap()
        for hp in range(NHP)
    ]

    with tile.TileContext(nc) as tc:
        with (
            tc.tile_pool(name="persist", bufs=1) as persist,
            tc.tile_pool(name="work", bufs=2) as work,
            tc.tile_pool(name="ps_st", bufs=4, space="PSUM") as ps_st,
            tc.tile_pool(name="ps_out", bufs=2, space="PSUM") as ps_out,
        ):
            # ---- persistent SBUF tiles
            ndc = [[persist.tile([128, 512], F16, tag=f"nd{fc}{jh}",
                                 name=f"nd{fc}{jh}")
                    for jh in range(2)] for fc in range(NFC)]
            wk_a = persist.tile([128, 256], F16, tag="wk_a")
            wk_b = persist.tile([128, 3 * 256], F16, tag="wk_b")
            wq_a = persist.tile([128, 256], F16, tag="wq_a")
            wq_b = persist.tile([128, 3 * 256], F16, tag="wq_b")
            wv = persist.tile([128, NFC * 256], F16, tag="wv")
            wo = persist.tile([128, NHP * FEAT], F16, tag="wo")
            bkq_sb = persist.tile([128, 4], F32, tag="bkq")
            # KT/QT split per j/i-half so the first scores start after the
            # first-half projections only (deps are tile-granular)
            KT = [[persist.tile([128, 512], F16, tag=f"kt{t}{jh}",
                                name=f"kt{t}{jh}") for jh in range(2)]
                  for t in range(NHP)]
            QT = [[persist.tile([128, 512], F16, tag=f"qt{t}{jh}",
                                name=f"qt{t}{jh}") for jh in range(2)]
                  for t in range(NHP)]
            # V' in four tiles (2 jt each) so each PV chunk waits only for
            # its own slice of the V projection
            V4 = [persist.tile([128, 2 * NH * VW], F16, tag=f"v4{q}",
                               name=f"v4{q}") for q in range(4)]
            v4r = [v.rearrange("p (j c) -> p j c", c=NH * VW) for v in V4]
            eb = [[persist.tile([128, 4 * N], F16, tag=f"eb{h}{hf}",
                                name=f"eb{h}{hf}")
                   for hf in range(2)] for h in range(NH)]
            OTn = [persist.tile([128, N], F16, tag=f"otn{hp}", name=f"otn{hp}")
                   for hp in range(NHP)]
            ones16 = persist.tile([128, 64], F16, tag="ones16")

            def wslice(a, b_, fc, t):
                # stationary [128, 128] slice for K/Q proj tile t, chunk fc
                w = a if fc == 0 else b_
                off = 0 if fc == 0 else (fc - 1) * 256
                return w[:, off + t * 128: off + (t + 1) * 128]

            # ---- input DMAs: first-half projection inputs first (they gate
            # the first scores), then second half, then eb per head in
            # consumption order
            nc.sync.dma_start(out=bkq_sb, in_=bkq)
            nc.scalar.dma_start(out=wk_a, in_=wk4[:, 0:256])
            nc.sync.dma_start(out=ndc[0][0], in_=ndT[0:128, 0:512])
            nc.scalar.dma_start(out=wq_a, in_=wq4[:, 0:256])
            nc.sync.dma_start(out=ndc[1][0], in_=ndT[128:256, 0:512])
            nc.scalar.dma_start(out=wk_b, in_=wk4[:, 256:1024])
            nc.sync.dma_start(out=ndc[2][0], in_=ndT[256:384, 0:512])
            nc.scalar.dma_start(out=wq_b, in_=wq4[:, 256:1024])
            nc.sync.dma_start(out=ndc[3][0], in_=ndT[384:512, 0:512])
            nc.scalar.dma_start(out=wv, in_=wv4)
            nc.scalar.dma_start(out=ndc[0][1], in_=ndT[0:128, 512:1024])
            nc.sync.dma_start(out=ndc[1][1], in_=ndT[128:256, 512:1024])
            nc.scalar.dma_start(out=ndc[2][1], in_=ndT[256:384, 512:1024])
            nc.sync.dma_start(out=ndc[3][1], in_=ndT[384:512, 512:1024])
            nc.scalar.dma_start(out=eb[0][0], in_=ebd[0][:, 0:4096])
            nc.sync.dma_start(out=eb[0][1], in_=ebd[0][:, 4096:8192])
            nc.scalar.dma_start(out=eb[1][0], in_=ebd[1][:, 0:4096])
            nc.sync.dma_start(out=eb[1][1], in_=ebd[1][:, 4096:8192])
            nc.sync.dma_start(out=wo, in_=wo4)
            nc.sync.dma_start(out=eb[2][0], in_=ebd[2][:, 0:4096])
            nc.sync.dma_start(out=eb[2][1], in_=ebd[2][:, 4096:8192])
            nc.sync.dma_start(out=eb[3][0], in_=ebd[3][:, 0:4096])
            nc.sync.dma_start(out=eb[3][1], in_=ebd[3][:, 4096:8192])

            nc.gpsimd.memset(ones16, 1.0)
            # ones columns of V' (col 64 of each head block)
            for q in range(4):
                for h in range(NH):
                    nc.gpsimd.memset(
                        v4r[q][:, :, h * VW + 64:h * VW + 65], 1.0)

            # ---- projections (first-half for both pairs, then second half)
            def emit_kqproj(t, jh):
                kq = ps_st.tile([128, 512], F32, tag="st1", name=f"kp{t}{jh}")
                qq = ps_st.tile([128, 512], F32, tag="st1", name=f"qp{t}{jh}")
                for fc in range(NFC):
                    nc.tensor.matmul(
                        kq, wslice(wk_a, wk_b, fc, t), ndc[fc][jh],
                        start=(fc == 0), stop=(fc == NFC - 1),
                    )
                nc.vector.tensor_scalar_add(KT[t][jh], kq, bkq_sb[:, t:t + 1])
                for fc in range(NFC):
                    nc.tensor.matmul(
                        qq, wslice(wq_a, wq_b, fc, t), ndc[fc][jh],
                        start=(fc == 0), stop=(fc == NFC - 1),
                    )
                nc.vector.tensor_scalar(
                    QT[t][jh], qq, bkq_sb[:, 2 + t:3 + t], SCALE,
                    op0=ALU.add, op1=ALU.mult,
                )

            def emit_vproj_pair(jp):
                ps = ps_st.tile([128, 512], F32, tag="st1", name=f"vp{jp}")
                for k in range(2):
                    jt = 2 * jp + k
                    sub = ps[:, k * 256:(k + 1) * 256]
                    for fc in range(NFC):
                        nc.tensor.matmul(
                            sub,
                            ndc[fc][jt // 4][:, (jt % 4) * 128:
                                             (jt % 4 + 1) * 128],
                            wv.rearrange("p (c d) -> p c d", d=256)[:, fc, :],
                            start=(fc == 0), stop=(fc == NFC - 1),
                        )
                    nc.vector.tensor_copy(
                        v4r[jp].rearrange(
                            "p j (h c) -> p j h c", c=VW)[:, k, :, 0:64],
                        sub.rearrange("p (h c) -> p h c", c=64),
                    )

            # ---- output projection chunk for one head pair (2 row chunks)
            def emit_oproj_chunk(hp, icp, tail=False):
                fps = [ps_st.tile([128, 512], F32, tag="st1",
                                  name=f"fp{hp}_{icp}_{k}") for k in range(2)]
                for k in range(2):
                    ic = icp * 2 + k
                    nc.tensor.matmul(
                        fps[k],
                        OTn[hp][:, ic * 128:(ic + 1) * 128],
                        wo[:, hp * FEAT:(hp + 1) * FEAT],
                        start=True, stop=True,
                    )
                for k in range(2):
                    ic = icp * 2 + k
                    osb = work.tile([128, 512], F16, tag="osb", bufs=4,
                                    name=f"osb{hp}_{ic}")
                    if tail:
                        nc.scalar.activation(osb, fps[k], AF.Copy)
                    else:
                        nc.vector.tensor_copy(osb, fps[k])
                    nc.sync.dma_start(
                        out=outs[hp][ic * 128:(ic + 1) * 128, :], in_=osb
                    )

            # ---- attention pair, software-pipelined in 2-jt windows so the
            # in-order PE queue always alternates scores (feeding ACT) with
            # PV work, and `filler` work slots into the PE slack
            def emit_pair(hp, filler, prelude=None):
                hA, hB = 2 * hp, 2 * hp + 1
                es = [work.tile([128, NJT * N], F16, tag="es", bufs=2,
                                name=f"es{h}") for h in (hA, hB)]
                et = [work.tile([128, NJT * N], F16, tag="et", bufs=2,
                                name=f"et{h}") for h in (hA, hB)]
                etr = [e.rearrange("p (j i) -> p j i", i=N) for e in et]
                outps = [ps_out.tile([128, 1024], F32, tag="outp",
                                     name=f"outp{h}") for h in (hA, hB)]

                def emit_scores(jt, ics=(0, 1)):
                    for ic in ics:
                        st = [ps_st.tile([128, 512], F32, tag="st1",
                                         name=f"st{hp}_{jt}_{ic}_{i}")
                              for i in range(2)]
                        for idx in range(2):
                            po = idx * 64
                            nc.tensor.matmul(
                                st[idx],
                                KT[hp][jt // 4][po:po + 64,
                                                (jt % 4) * 128:
                                                (jt % 4 + 1) * 128],
                                QT[hp][ic][po:po + 64, :],
                                start=True, stop=True,
                            )
                        for idx in range(2):
                            nc.scalar.activation(
                                es[idx][:, jt * N + ic * 512:
                                        jt * N + ic * 512 + 512],
                                st[idx], AF.Exp
                            )

                def emit_mult(jp):
                    for idx, h in enumerate((hA, hB)):
                        ebs = eb[h][jp // 2][:, (jp % 2) * 2048:
                                             (jp % 2) * 2048 + 2048]
                        ess = es[idx][:, jp * 2 * N:(jp * 2 + 2) * N]
                        ets = et[idx][:, jp * 2 * N:(jp * 2 + 2) * N]
                        nc.vector.tensor_mul(ets, ess, ebs)

                def emit_pv(jp):
                    order = ((1, hB), (0, hA)) if jp == 0 else \
                        ((0, hA), (1, hB))
                    for idx, h in order:
                        for ic in range(2):
                            for jt in (2 * jp, 2 * jp + 1):
                                nc.tensor.matmul(
                                    outps[idx][0:65, ic * 512:(ic + 1) * 512],
                                    v4r[jp][:, jt % 2,
                                            h * VW:h * VW + 65],
                                    etr[idx][:, jt, ic * 512:(ic + 1) * 512],
                                    start=(jp == 0 and jt == 0),
                                    stop=(jp == 3 and jt == NJT - 1),
                                    skip_group_check=True,
                                )

                if prelude is not None:
                    # ic0 scores of the first two key chunks depend only on
                    # the first projection tiles -> the exp stream starts
                    # while the second projection (prelude) still runs
                    emit_scores(0, ics=(0,))
                    emit_scores(1, ics=(0,))
                    prelude()
                    emit_scores(0, ics=(1,))
                    emit_scores(1, ics=(1,))
                for w in range(4):
                    for k in range(2):
                        if not (prelude is not None and w == 0):
                            emit_scores(2 * w + k)
                    if filler:
                        f = filler.pop(0)
                        if f is not None:
                            f()
                    if w >= 1:
                        emit_pv(w - 1)
                    emit_mult(w)
                emit_pv(3)

                # normalization, deferred + split: pre = PSUM eviction and
                # the Ln/Exp reciprocal (ACT/DVE); post = broadcast matmuls
                # and the normalize multiply (PE/DVE). The caller slots these
                # into the next pair's windows so neither the ACT nor the PE
                # queue head-blocks at pair boundaries.
                state = {}

                def norm_pre(idx, h):
                    outp = outps[idx]
                    otd = work.tile([64, N], F32, tag="otd", name=f"otd{h}")
                    nc.vector.tensor_copy(otd, outp[0:64, :])
                    lnv = work.tile([65, N], F32, tag="lnv", name=f"lnv{h}")
                    nc.scalar.activation(lnv[64:65, :], outp[64:65, :], AF.Ln)
                    rec16 = work.tile([65, N], F16, tag="rec16",
                                      name=f"rec16{h}")
                    nc.scalar.activation(rec16[64:65, :], lnv[64:65, :],
                                         AF.Exp, scale=-1.0)
                    state[idx] = (otd, rec16)

                def norm_post(idx, h):
                    otd, rec16 = state[idx]
                    rbcs = [ps_st.tile([128, 512], F32, tag="st1",
                                       name=f"rbc{h}_{ic}") for ic in range(2)]
                    for ic in range(2):
                        nc.tensor.matmul(
                            rbcs[ic][0:64, :],
                            ones16[64:65, :],
                            rec16[64:65, ic * 512:(ic + 1) * 512],
                            start=True, stop=True,
                        )
                        nc.vector.tensor_mul(
                            OTn[hp][idx * 64:idx * 64 + 64,
                                    ic * 512:(ic + 1) * 512],
                            otd[:, ic * 512:(ic + 1) * 512],
                            rbcs[ic][0:64, :],
                        )

                return [lambda: (norm_pre(0, hA), norm_pre(1, hB)),
                        lambda: (norm_post(0, hA), norm_post(1, hB))]

            # ---- schedule: minimal pre-pair projections; V'/remaining
            # projections/previous norms/oproj ride the pair windows
            emit_kqproj(0, 0)

            norm0 = emit_pair(0, [
                lambda: emit_vproj_pair(0),
                lambda: emit_vproj_pair(1),
                lambda: emit_vproj_pair(2),
                lambda: (emit_vproj_pair(3), emit_kqproj(1, 0),
                         emit_kqproj(1, 1)),
            ], prelude=lambda: emit_kqproj(0, 1))
            norm1 = emit_pair(1, [
                None,
                norm0[0],
                lambda: (norm0[1](), emit_oproj_chunk(0, 0)),
                lambda: (emit_oproj_chunk(0, 1), emit_oproj_chunk(0, 2)),
            ])
            emit_oproj_chunk(0, 3)
            for nrm in norm1:
                nrm()
            for icp in range(4):
                emit_oproj_chunk(1, icp, tail=True)

    nc.compile()
    return nc


def _prep_inputs(ndata, attn_bias, attn_mask, Wq, bq, Wk, bk, Wv, bv, Wo, bo):
    ndata = np.asarray(ndata, dtype=np.float32)
    attn_bias = np.asarray(attn_bias, dtype=np.float32)
    attn_mask = np.asarray(attn_mask)
    Wq, Wk, Wv, Wo = (np.asarray(w, dtype=np.float32) for w in (Wq, Wk, Wv, Wo))
    bq, bk, bv, bo = (np.asarray(v, dtype=np.float32) for v in (bq, bk, bv, bo))

    ebf = np.where(attn_mask, np.float32(0.0), np.exp(attn_bias))  # [B,N,N,H]

    wqT = Wq.T.astype(np.float16)  # [in_f, out_f]
    wkT = Wk.T.astype(np.float16)
    wvT = Wv.T.astype(np.float16)
    woT = Wo.T.astype(np.float16)

    in_maps = []
    for core in range(N_CORES):
        b, hh = core // 2, core % 2
        cs = hh * 256  # feature-col offset of this core's 4 heads
        wk4 = np.ascontiguousarray(
            wkT[:, cs:cs + 256].reshape(4, 128, 256).transpose(1, 0, 2)
            .reshape(128, 1024))
        wq4 = np.ascontiguousarray(
            wqT[:, cs:cs + 256].reshape(4, 128, 256).transpose(1, 0, 2)
            .reshape(128, 1024))
        wv4 = np.ascontiguousarray(
            wvT[:, cs:cs + 256].reshape(4, 128, 256).transpose(1, 0, 2)
            .reshape(128, 1024))
        wo4 = np.ascontiguousarray(
            woT[cs:cs + 256, :].reshape(2, 128, 512).transpose(1, 0, 2)
            .reshape(128, 1024))
        bkq = np.ascontiguousarray(np.concatenate(
            [bk[cs:cs + 256].reshape(2, 128).T,
             bq[cs:cs + 256].reshape(2, 128).T], axis=1)).astype(np.float32)
        arr = ebf[b][:, :, hh * 4:hh * 4 + 4]        # [i, j, hl]
        arr = arr.transpose(2, 1, 0)                 # [hl, j, i]
        arr = arr.reshape(4, 8, 128, N).transpose(0, 2, 1, 3)  # [hl,p,jt,i]
        ebc = np.ascontiguousarray(arr.reshape(4, 128, 8192).astype(np.float16))
        in_maps.append({
            "ndT": np.ascontiguousarray(ndata[b].T.astype(np.float16)),
            "wk4": wk4, "wq4": wq4, "wv4": wv4, "wo4": wo4,
            "bkq": bkq, "ebd": ebc,
        })
    return in_maps


def kernel(ndata, attn_bias, attn_mask, Wq, bq, Wk, bk, Wv, bv, Wo, bo,
           _trace=False):
    if "nc" not in _CACHE:
        _CACHE["nc"] = _build()
    nc = _CACHE["nc"]
    in_maps = _prep_inputs(ndata, attn_bias, attn_mask, Wq, bq, Wk, bk, Wv, bv,
                           Wo, bo)
    res = run_bass_kernel_spmd(nc, in_maps, list(range(N_CORES)), trace=_trace)
    _CACHE["last_res"] = res
    const = (np.asarray(bo, dtype=np.float32)
             + np.asarray(bv, dtype=np.float32)
             @ np.asarray(Wo, dtype=np.float32).T)
    full = np.empty((B, N, FEAT), dtype=np.float32)
    for b in range(B):
        acc = np.zeros((N, FEAT), dtype=np.float32)
        for hh in range(2):
            r = res.results[2 * b + hh]
            acc += r["out0"].astype(np.float32)
            acc += r["out1"].astype(np.float32)
        full[b] = acc + const[None, :]
    return full
